# revision 13
# baseline (speedup 1.0000x reference)
"""AdaptiveFrequencyAsymmetricHuberLoss on 8 TRN2 NeuronCores (Bass/Tile).

loss = mean( wf(t) * asym(t, sign(e)) * huber(e, delta(t)) ),  e = p - t
  delta(t)   = 5 + 0.05 t
  w_under(t) = 1 + 0.05 t
  w_over(t)  = 2 exp(-t/10)
  wf(t)      = clip(3 / (freq[t] + 1), 1, 3)   (t integer 0..130)
  huber      = 0.5 cl (2e - cl), cl = clip(e, -delta, delta)   (exact identity)

Sharding: pure data parallel; each of the 8 cores streams a contiguous
1/8 of the elements as [128, 16384], DMA-cast f32->bf16 on load.

Per-tile pipeline (engines balanced against the ~47 us/core HBM read):
  ACT:  nd = -delta, ws = w_over (Exp), wu = w_under     [3 passes]
  DVE:  e = p - t                                        [bf16 2x]
        h0 = cl*(2e-cl) = 2*huber   [one fused custom-DVE op, 1x]
        mk = (e < 0)                                     [bf16 4x]
        copy_predicated(ws, mk, wu)  -> asym weight      [2x]
        q = (ws * 0.5) * h0, accum_out -> partial sums   [2x]

The freq table is handled host-side: wf >= 1 always, and wf > 1 only
for freq counts < 2, so the host enumerates the (usually zero) table
entries with wf > 1 and the kernel adds one masked correction pass per
entry. Per-partition partial sums return via accum_out; host reduces
in float64 and divides by N.
"""

import numpy as np

import concourse.bass as bass
import concourse.dve_ops as dve_ops_mod
import concourse.tile as tile
from concourse import bacc, mybir
from concourse.bass_utils import run_bass_kernel_spmd
from concourse.dve_ops import DveOp
from concourse.dve_spec import (
    Spec,
    Src0,
    Src1,
    Zero,
    _has_src1,
    lower,
    maxx,
    minn,
)
from concourse.dve_uop import DveOpSpec

N = 16_777_216
NCORES = 8
P = 128
PER_CORE = N // NCORES          # 2_097_152
FREE = PER_CORE // P            # 16384
TILE_F = 4096
NTILES = FREE // TILE_F         # 4

LN2 = 0.6931471805599453

f32 = mybir.dt.float32
bf16 = mybir.dt.bfloat16
u16 = mybir.dt.uint16


def _register_op(name, spec):
    for o in dve_ops_mod.OPS:
        if o.name == name:
            return o
    opcode = max(dve_ops_mod._SUB_OPCODE_FOR_NAME.values()) + 1
    assert opcode < 0x20, "custom-DVE opcode rows exhausted"
    shas = {}
    for ver in ("v3", "v4"):
        try:
            c = DveOpSpec(
                name=name, opcode=opcode, uops=lower(spec, ver=ver),
                rd1_en=_has_src1(spec),
            )
            shas[ver] = c.sha(ver)
        except Exception:
            pass
    op = DveOp(name, spec, subdim=False, uops_sha=shas)
    dve_ops_mod.OPS.append(op)
    dve_ops_mod.CUSTOM_DVE_SPECS[name] = spec
    dve_ops_mod._SUB_OPCODE_FOR_NAME[name] = opcode
    return op


def _huber2_ref(in0, in1, c0, c1, c2):
    e = in0.astype(np.float32)
    nd = in1.astype(np.float32)
    cl = np.minimum(np.maximum(e, nd), -nd)
    return (cl * ((e + e) - cl)).astype(np.float32)


# h0 = cl*(2e - cl) = 2*huber(e, delta);  in0 = e, in1 = nd = -delta
_dd = Zero - Src1
_cl = minn(maxx(Src0, Src1), _dd)
HUBER2_SPEC = Spec(
    body=_cl * ((Src0 + Src0) - _cl),
    reference=_huber2_ref,
)

HUBER2_OP = _register_op("HUBER2_LOSS_ANT", HUBER2_SPEC)


def build(corrections):
    """Build + compile the SPMD graph. corrections: tuple of (k, wf_k - 1)."""
    Alu = mybir.AluOpType
    Act = mybir.ActivationFunctionType

    nc = bacc.Bacc(
        "TRN2", target_bir_lowering=False, debug=False, num_devices=NCORES
    )

    # const APs for activation biases (only 0.0/1.0 are pre-registered)
    for val in (-5.0, LN2):
        h = nc.alloc_sbuf_tensor(f"const-f32-{val}", [128, 1], f32)
        nc.gpsimd.memset(h.ap(), val)
        nc.const_aps.aps[(f32, val)] = h.ap()
    nc.all_engine_barrier()

    p_ap = nc.dram_tensor("p", [P, FREE], f32, kind="ExternalInput").ap()
    t_ap = nc.dram_tensor("t", [P, FREE], f32, kind="ExternalInput").ap()
    ncols = NTILES * (1 + len(corrections))
    o_ap = nc.dram_tensor("out", [P, ncols], f32, kind="ExternalOutput").ap()

    with tile.TileContext(nc) as tc:
        with (
            tc.tile_pool(name="io", bufs=3) as io_pool,
            tc.tile_pool(name="tmp", bufs=2) as tmp,
            tc.tile_pool(name="acc", bufs=1) as acc_pool,
        ):
            accs = acc_pool.tile([P, ncols], f32)
            col = 0
            for i in range(NTILES):
                sl = slice(i * TILE_F, (i + 1) * TILE_F)
                pt = io_pool.tile([P, TILE_F], bf16)
                nc.gpsimd.dma_start(out=pt[:], in_=p_ap[:, sl])  # f32->bf16
                tt = io_pool.tile([P, TILE_F], bf16)
                nc.gpsimd.dma_start(out=tt[:], in_=t_ap[:, sl])

                nd = tmp.tile([P, TILE_F], bf16)  # -delta = -5 - 0.05 t
                nc.scalar.activation(nd[:], tt[:], Act.Identity, bias=-5.0, scale=-0.05)
                ws = tmp.tile([P, TILE_F], bf16)  # w_over = exp(-0.1 t + ln 2)
                nc.scalar.activation(ws[:], tt[:], Act.Exp, bias=LN2, scale=-0.1)
                wu = tmp.tile([P, TILE_F], bf16)  # w_under = 1 + 0.05 t
                nc.scalar.activation(wu[:], tt[:], Act.Identity, bias=1.0, scale=0.05)

                e = tmp.tile([P, TILE_F], bf16)
                nc.vector.tensor_tensor(out=e[:], in0=pt[:], in1=tt[:], op=Alu.subtract)
                h0 = tmp.tile([P, TILE_F], bf16)  # 2*huber
                nc.vector._custom_dve(HUBER2_OP, out=h0[:], in0=e[:], in1=nd[:])
                mk = tmp.tile([P, TILE_F], bf16)  # 1.0 iff e < 0
                nc.vector.tensor_scalar(
                    out=mk[:], in0=e[:], scalar1=0.0, scalar2=None, op0=Alu.is_lt
                )
                # asym weight: w_over, overwritten with w_under where e < 0
                nc.vector.copy_predicated(
                    out=ws[:], mask=mk[:].bitcast(u16), data=wu[:]
                )
                # q = (w * 0.5) * h0 ; accumulate per-partition sums
                q = tmp.tile([P, TILE_F], bf16)
                nc.vector.scalar_tensor_tensor(
                    out=q[:], in0=ws[:], scalar=0.5, in1=h0[:],
                    op0=Alu.mult, op1=Alu.mult,
                    accum_out=accs[:, col : col + 1],
                )
                col += 1
                for k, dw in corrections:
                    ck = tmp.tile([P, TILE_F], bf16)
                    nc.vector.tensor_scalar(
                        out=ck[:], in0=tt[:], scalar1=float(k), scalar2=None,
                        op0=Alu.is_equal,
                    )
                    qc = tmp.tile([P, TILE_F], bf16)
                    nc.vector.scalar_tensor_tensor(
                        out=qc[:], in0=ck[:], scalar=float(dw), in1=q[:],
                        op0=Alu.mult, op1=Alu.mult,
                        accum_out=accs[:, col : col + 1],
                    )
                    col += 1
            nc.sync.dma_start(out=o_ap[:], in_=accs[:])
    nc.compile()
    return nc


_cache = {}


def get_nc(corrections):
    key = tuple(corrections)
    if key not in _cache:
        _cache[key] = build(key)
    return _cache[key]


def make_in_maps(predictions, targets):
    p = np.ascontiguousarray(np.asarray(predictions, dtype=np.float32)).reshape(
        NCORES, P, FREE
    )
    t = np.ascontiguousarray(np.asarray(targets, dtype=np.float32)).reshape(
        NCORES, P, FREE
    )
    return [{"p": p[c], "t": t[c]} for c in range(NCORES)]


def freq_corrections(freq_counts):
    fc = np.asarray(freq_counts, dtype=np.float32)
    wf = np.clip(
        np.float32(3.0) / (fc + np.float32(1.0)), np.float32(1.0), np.float32(3.0)
    )
    ks = np.nonzero(wf > 1.0)[0]
    return tuple((int(k), float(wf[k] - 1.0)) for k in ks)


def _run(in_maps, corrections, **kwargs):
    nc = get_nc(corrections)
    return run_bass_kernel_spmd(nc, in_maps, core_ids=list(range(NCORES)), **kwargs)


def kernel(predictions, targets, freq_counts):
    corrections = freq_corrections(freq_counts)
    in_maps = make_in_maps(predictions, targets)
    res = _run(in_maps, corrections)
    total = np.float64(0.0)
    for c in range(NCORES):
        total += np.asarray(res.results[c]["out"], dtype=np.float64).sum()
    return np.array(total / N, dtype=np.float32)


# revision 16
# speedup vs baseline: 1.1377x; 1.1377x over previous
"""AdaptiveFrequencyAsymmetricHuberLoss on 8 TRN2 NeuronCores (Bass/Tile).

loss = mean( wf(t) * asym(t, sign(e)) * huber(e, delta(t)) ),  e = p - t
  delta(t)   = 5 + 0.05 t
  w_under(t) = 1 + 0.05 t
  w_over(t)  = 2 exp(-t/10)
  wf(t)      = clip(3 / (freq[t] + 1), 1, 3)   (t integer 0..130)
  huber      = 0.5 cl (2e - cl), cl = clip(e, -delta, delta)   (exact identity)

Sharding: pure data parallel; each of the 8 cores streams a contiguous
1/8 of the elements as [128, 16384], DMA-cast f32->bf16 on load.

Per-tile pipeline:
  ACT:  nd = -delta,  ws = w_over (Exp)
  DVE:  e  = p - t                                   [bf16 2x]
        sh = |cl| * (2e - cl) = sign(e) * 2*huber    [8-op custom, 1x]
        shp = max(sh, 0), rm = max(-sh, 0)           [bf16 4x]
        wu = 1 + 0.05 t                              [bf16 4x]
        qo = shp * ws,  qu = rm * wu                 [bf16 2x]
  PE:   ones-colsum matmuls accumulate sum(qo)+sum(qu) into one
        [1,512] PSUM bank across all tiles (only the total matters).
Host divides by 2N and reduces in float64.

The freq table is handled host-side: wf >= 1 always, and wf > 1 only
for freq counts < 2, so the host enumerates the (usually zero) table
entries with wf > 1 and the kernel adds masked correction passes per
entry (accum_out into a separate SBUF accumulator).
"""

import contextlib

import numpy as np

import concourse.bass as bass
import concourse.dve_ops as dve_ops_mod
import concourse.tile as tile
from concourse import bacc, mybir
from concourse.bass_utils import run_bass_kernel_spmd
from concourse.dve_ops import DveOp
from concourse.dve_spec import (
    Spec,
    Src0,
    Src1,
    Zero,
    _has_src1,
    lower,
    maxx,
    minn,
)
from concourse.dve_uop import DveOpSpec

N = 16_777_216
NCORES = 8
P = 128
PER_CORE = N // NCORES          # 2_097_152
FREE = PER_CORE // P            # 16384
TILE_FS = [4096, 4096, 4096, 4096]
assert sum(TILE_FS) == FREE

LN2 = 0.6931471805599453

f32 = mybir.dt.float32
bf16 = mybir.dt.bfloat16


def _register_op(name, spec):
    for o in dve_ops_mod.OPS:
        if o.name == name:
            return o
    opcode = max(dve_ops_mod._SUB_OPCODE_FOR_NAME.values()) + 1
    assert opcode < 0x20, "custom-DVE opcode rows exhausted"
    shas = {}
    for ver in ("v3", "v4"):
        try:
            c = DveOpSpec(
                name=name, opcode=opcode, uops=lower(spec, ver=ver),
                rd1_en=_has_src1(spec),
            )
            shas[ver] = c.sha(ver)
        except Exception:
            pass
    op = DveOp(name, spec, subdim=False, uops_sha=shas)
    dve_ops_mod.OPS.append(op)
    dve_ops_mod.CUSTOM_DVE_SPECS[name] = spec
    dve_ops_mod._SUB_OPCODE_FOR_NAME[name] = opcode
    return op


def _huber_signed_ref(in0, in1, c0, c1, c2):
    e = in0.astype(np.float32)
    nd = in1.astype(np.float32)
    cl = np.minimum(np.maximum(e, nd), -nd)
    return (np.abs(cl) * ((e + e) - cl)).astype(np.float32)


# sh = |cl| * (2e - cl) = sign(e) * 2*huber(e, delta);  in0 = e, in1 = -delta
_dd = Zero - Src1
_cl = minn(maxx(Src0, Src1), _dd)
_v = (Src0 + Src0) - _cl
_acl = maxx(_cl, Zero - _cl)
HUBER_SIGNED_SPEC = Spec(
    body=_acl * _v,
    reference=_huber_signed_ref,
)

HUBER_SIGNED_OP = _register_op("HUBER_SIGNED_LOSS_ANT", HUBER_SIGNED_SPEC)


def build(corrections):
    """Build + compile the SPMD graph. corrections: tuple of (k, wf_k - 1)."""
    Alu = mybir.AluOpType
    Act = mybir.ActivationFunctionType

    nc = bacc.Bacc(
        "TRN2", target_bir_lowering=False, debug=False, num_devices=NCORES
    )

    # const APs for activation biases (only 0.0/1.0 are pre-registered)
    for val in (-5.0, LN2):
        h = nc.alloc_sbuf_tensor(f"const-f32-{val}", [128, 1], f32)
        nc.gpsimd.memset(h.ap(), val)
        nc.const_aps.aps[(f32, val)] = h.ap()
    ones_h = nc.alloc_sbuf_tensor("ones-bf16", [128, 1], bf16)
    nc.gpsimd.memset(ones_h.ap(), 1.0)
    ones = ones_h.ap()
    nc.all_engine_barrier()

    p_ap = nc.dram_tensor("p", [P, FREE], f32, kind="ExternalInput").ap()
    t_ap = nc.dram_tensor("t", [P, FREE], f32, kind="ExternalInput").ap()
    o_ap = nc.dram_tensor("out", [1, 512], f32, kind="ExternalOutput").ap()
    oc_ap = None
    if corrections:
        oc_ap = nc.dram_tensor(
            "outc", [P, len(TILE_FS) * len(corrections)], f32,
            kind="ExternalOutput",
        ).ap()

    n_mms = 0
    total_mms = sum(2 * (f // 512) for f in TILE_FS)

    with contextlib.ExitStack() as es:
        tc = es.enter_context(tile.TileContext(nc))
        io_pool = es.enter_context(tc.tile_pool(name="io", bufs=3))
        tmp = es.enter_context(tc.tile_pool(name="tmp", bufs=2))
        ps_pool = es.enter_context(
            tc.tile_pool(name="ps", bufs=1, space=bass.MemorySpace.PSUM)
        )
        acc_pool = es.enter_context(tc.tile_pool(name="acc", bufs=1))

        psum = ps_pool.tile([1, 512], f32)
        accs = None
        if corrections:
            accs = acc_pool.tile([P, len(TILE_FS) * len(corrections)], f32)

        def colsum(src_ap, tf):
            nonlocal n_mms
            for c in range(0, tf, 512):
                nc.tensor.matmul(
                    psum[:], ones, src_ap[:, c : c + 512],
                    start=(n_mms == 0), stop=(n_mms == total_mms - 1),
                )
                n_mms += 1

        col = 0
        off = 0
        for i, TF in enumerate(TILE_FS):
            sl = slice(off, off + TF)
            off += TF
            pt = io_pool.tile([P, TF], bf16, tag="pt")
            nc.gpsimd.dma_start(out=pt[:], in_=p_ap[:, sl])  # f32->bf16
            tt = io_pool.tile([P, TF], bf16, tag="tt")
            nc.gpsimd.dma_start(out=tt[:], in_=t_ap[:, sl])

            nd = tmp.tile([P, TF], bf16, tag="nd")  # -delta = -5 - 0.05 t
            nc.scalar.activation(nd[:], tt[:], Act.Identity, bias=-5.0, scale=-0.05)
            ws = tmp.tile([P, TF], bf16, tag="ws")  # w_over = exp(-0.1 t + ln 2)
            nc.scalar.activation(ws[:], tt[:], Act.Exp, bias=LN2, scale=-0.1)
            wu = tmp.tile([P, TF], bf16, tag="wu")  # w_under = 1 + 0.05 t
            nc.vector.tensor_scalar(
                out=wu[:], in0=tt[:], scalar1=0.05, scalar2=1.0,
                op0=Alu.mult, op1=Alu.add,
            )

            e = tmp.tile([P, TF], bf16, tag="e")
            nc.vector.tensor_tensor(out=e[:], in0=pt[:], in1=tt[:], op=Alu.subtract)
            sh = tmp.tile([P, TF], bf16, tag="sh")  # sign(e) * 2*huber
            nc.vector._custom_dve(HUBER_SIGNED_OP, out=sh[:], in0=e[:], in1=nd[:])
            shp = tmp.tile([P, TF], bf16, tag="shp")  # 2*huber where e>0
            nc.vector.tensor_scalar(
                out=shp[:], in0=sh[:], scalar1=0.0, scalar2=None, op0=Alu.max
            )
            rm = tmp.tile([P, TF], bf16, tag="rm")  # 2*huber where e<0
            nc.vector.tensor_scalar(
                out=rm[:], in0=sh[:], scalar1=-1.0, scalar2=0.0,
                op0=Alu.mult, op1=Alu.max,
            )
            qo = tmp.tile([P, TF], bf16, tag="qo")
            nc.vector.tensor_tensor(out=qo[:], in0=shp[:], in1=ws[:], op=Alu.mult)
            qu = tmp.tile([P, TF], bf16, tag="qu")
            nc.vector.tensor_tensor(out=qu[:], in0=rm[:], in1=wu[:], op=Alu.mult)
            colsum(qo, TF)
            colsum(qu, TF)

            for k, dw in corrections:
                qd = tmp.tile([P, TF], bf16, tag="qd")
                nc.vector.tensor_tensor(out=qd[:], in0=qo[:], in1=qu[:], op=Alu.add)
                ck = tmp.tile([P, TF], bf16, tag="ck")
                nc.vector.tensor_scalar(
                    out=ck[:], in0=tt[:], scalar1=float(k), scalar2=None,
                    op0=Alu.is_equal,
                )
                qc = tmp.tile([P, TF], bf16, tag="qc")
                nc.vector.scalar_tensor_tensor(
                    out=qc[:], in0=ck[:], scalar=float(dw), in1=qd[:],
                    op0=Alu.mult, op1=Alu.mult,
                    accum_out=accs[:, col : col + 1],
                )
                col += 1
        osb = acc_pool.tile([1, 512], f32, tag="osb")
        nc.vector.tensor_copy(out=osb[:], in_=psum[:])
        nc.sync.dma_start(out=o_ap[:], in_=osb[:])
        if corrections:
            nc.sync.dma_start(out=oc_ap[:], in_=accs[:])
    nc.compile()
    return nc


_cache = {}


def get_nc(corrections):
    key = tuple(corrections)
    if key not in _cache:
        _cache[key] = build(key)
    return _cache[key]


def make_in_maps(predictions, targets):
    p = np.ascontiguousarray(np.asarray(predictions, dtype=np.float32)).reshape(
        NCORES, P, FREE
    )
    t = np.ascontiguousarray(np.asarray(targets, dtype=np.float32)).reshape(
        NCORES, P, FREE
    )
    return [{"p": p[c], "t": t[c]} for c in range(NCORES)]


def freq_corrections(freq_counts):
    fc = np.asarray(freq_counts, dtype=np.float32)
    wf = np.clip(
        np.float32(3.0) / (fc + np.float32(1.0)), np.float32(1.0), np.float32(3.0)
    )
    ks = np.nonzero(wf > 1.0)[0]
    return tuple((int(k), float(wf[k] - 1.0)) for k in ks)


def _run(in_maps, corrections, **kwargs):
    nc = get_nc(corrections)
    return run_bass_kernel_spmd(nc, in_maps, core_ids=list(range(NCORES)), **kwargs)


def reduce_results(res, corrections):
    total = np.float64(0.0)
    for c in range(NCORES):
        total += np.asarray(res.results[c]["out"], dtype=np.float64).sum()
        if corrections:
            total += np.asarray(res.results[c]["outc"], dtype=np.float64).sum()
    return np.array(total / (2.0 * N), dtype=np.float32)


def kernel(predictions, targets, freq_counts):
    corrections = freq_corrections(freq_counts)
    in_maps = make_in_maps(predictions, targets)
    res = _run(in_maps, corrections)
    return reduce_results(res, corrections)
